# revision 9
# baseline (speedup 1.0000x reference)
"""BiLSTM (B=64, L=256, D=512, H=512) on 8 Trainium2 NeuronCores.

Sharding: 8 cores = 2 directions x 4 batch-slices of 16 (weights replicated
per direction, sequential time loop local to each core).  Backward-direction
cores receive time-reversed x, so every core runs the identical SPMD program.

v2 design (vs the fp32 baseline):
  - all matmuls in bf16: 1 cycle/moving-row instead of 4, single-pass
    (fp32 matmuls are issued by HW as 2 half-rate passes).
  - gate layout: one PSUM bank [128, 512] per step holds the 4 gates as
    column blocks of 128 (order [g, i, f, o]); each block is [4 h-chunks
    x 32-partition groups, 128 free], i.e. partition = 32*hc + b, free =
    h % 128.  Elementwise tail ops then process 128 free elements
    instead of 512 -> ~4x less serial ACT/DVE time per step.
  - x-part (x_t @ Wx + bias) precomputed as a full-utilization GEMM into
    SBUF (bf16) -- no DRAM round-trip -- and injected into PSUM with
    identity-stationary matmuls at the head of each step.
  - per step a single PE transpose of h [112, 128] yields all 4 next-step
    stationary chunks hT[k] as column slices; one DVE copy converts to
    bf16 stationary tile.
  - per-step PE order: inject(t) | transpose(t-1) | h-matmuls(t) so the
    injection overlaps the previous step's ACT/DVE tail.
"""

import numpy as np
import ml_dtypes

from concourse import tile, mybir, bacc
from concourse.bass_utils import run_bass_kernel_spmd
from concourse.masks import make_identity

FP = mybir.dt.float32
BF = mybir.dt.bfloat16
AF = mybir.ActivationFunctionType

B = 16        # local batch per core
L = 256       # timesteps
D = 512       # input dim
H = 512       # hidden
NG = 4 * H    # gate width
TOK = L * B   # tokens per core
NM = TOK // 128  # 32 phase-1 token tiles

# gate order on device: [g, i, f, o]
GATES = "gifo"

_CACHED_NC = None


def _build():
    nc = bacc.Bacc("TRN2", target_bir_lowering=False, debug=False)

    xT = nc.dram_tensor("xT", [D, TOK], BF, kind="ExternalInput").ap()
    Wx = nc.dram_tensor("Wx", [D, NG], BF, kind="ExternalInput").ap()
    Wh = nc.dram_tensor("Wh", [H, NG], BF, kind="ExternalInput").ap()
    bias = nc.dram_tensor("bias", [1, NG], BF, kind="ExternalInput").ap()
    out_h = nc.dram_tensor("out_h", [L, 112, 128], BF, kind="ExternalOutput").ap()

    with tile.TileContext(nc, trace_sim=False) as tc:
        with tc.tile_pool(name="wpool", bufs=1) as wpool, \
             tc.tile_pool(name="cpool", bufs=1) as cpool, \
             tc.tile_pool(name="xppool", bufs=1) as xppool:
            # weights, bias, identities (persistent)
            Wx_t, Wh_t = [], []
            for k in range(4):
                wt = wpool.tile([128, NG], BF, tag=f"wx{k}", name=f"wx{k}")
                nc.sync.dma_start(wt[:], Wx[128 * k:128 * (k + 1), :])
                Wx_t.append(wt)
            for k in range(4):
                wt = wpool.tile([128, NG], BF, tag=f"wh{k}", name=f"wh{k}")
                nc.sync.dma_start(wt[:], Wh[128 * k:128 * (k + 1), :])
                Wh_t.append(wt)
            bias_t = wpool.tile([1, NG], BF)
            nc.sync.dma_start(bias_t[:], bias[:, :])
            ones_t = cpool.tile([1, 128], BF)
            nc.vector.memset(ones_t[:, :], 1.0)
            ident = cpool.tile([128, 128], BF)
            make_identity(nc, ident[:, :])

            # persistent x-part buffer: 32 tiles [128 tokens, 2048] bf16
            # [token, G, hc, hl] -- G-major gate column order
            xp_sb = []
            for m in range(NM):
                xp_sb.append(
                    xppool.tile([128, 4, 4, 128], BF, tag=f"xp{m}", name=f"xp{m}"))

            # ---- Phase 1: xp = x @ Wx + bias (full PE utilization) ----
            with tc.tile_pool(name="p1x", bufs=3) as p1x, \
                 tc.tile_pool(name="p1ps", bufs=4, space="PSUM") as p1ps:
                for m in range(NM):
                    xm = p1x.tile([128, 4, 128], BF, tag="xm", name="xm")
                    for k in range(4):
                        nc.sync.dma_start(
                            xm[:, k, :],
                            xT[128 * k:128 * (k + 1), 128 * m:128 * (m + 1)])
                    for n in range(4):
                        ps = p1ps.tile([128, 512], FP, tag="ps1", name="ps1")
                        for k in range(4):
                            nc.tensor.matmul(
                                ps[:, :], xm[:, k, :],
                                Wx_t[k][:, 512 * n:512 * (n + 1)],
                                start=(k == 0), stop=False)
                        nc.tensor.matmul(
                            ps[:, :], ones_t[:, :],
                            bias_t[:, 512 * n:512 * (n + 1)],
                            start=False, stop=True)
                        # copy psum -> sbuf bf16, alternating engines
                        dst = xp_sb[m][:, n, :, :]
                        if (m * 4 + n) % 2 == 0:
                            nc.scalar.copy(dst, ps[:, :])
                        else:
                            nc.vector.tensor_copy(dst, ps[:, :])

            # ---- Phase 2: recurrence ----
            with tc.tile_pool(name="st", bufs=2) as st, \
                 tc.tile_pool(name="ch", bufs=2) as ch, \
                 tc.tile_pool(name="gps", bufs=2, space="PSUM") as gps, \
                 tc.tile_pool(name="tps", bufs=2, space="PSUM") as tps:

                # zero-init state; pre-zero both gate-psum ring buffers so
                # the never-written junk partition rows (16:32, 48:64,
                # 80:96) hold finite values for the full-height ACT ops.
                c_prev = st.tile([112, 128], FP, tag="c", name="c0")
                nc.vector.memset(c_prev[:, :], 0.0)
                hTs_prev = st.tile([128, 112], BF, tag="hTs", name="hTs0")
                nc.vector.memset(hTs_prev[:, :], 0.0)
                for z in range(2):
                    Pz = gps.tile([128, 512], FP, tag="P", name=f"Pz{z}")
                    nc.vector.memset(Pz[:, :], 0.0)

                h_prev = None
                for t in range(L):
                    m, r = divmod(t, 8)
                    P = gps.tile([128, 512], FP, tag="P", name="P")

                    # inject x-part (independent of h -> overlaps prev tail).
                    # moving-operand partition windows must be 32-aligned, so
                    # read a 32-row window and select rows with a shifted
                    # identity stationary: rows [16r .. 16r+16) of the m-tile
                    # live at window 32*(r//2), sub-offset 16*(r%2).
                    # NOTE: start=True clears has_written for the whole
                    # 32-partition column group of the bank, so there must be
                    # exactly ONE start mm per column group: each inject
                    # covers all 4 gate blocks (N=512) for its hc group.
                    w0 = 32 * (r // 2)
                    so = B * (r % 2)
                    for hc in range(4):
                        nc.tensor.matmul(
                            P[32 * hc:32 * hc + B, 0:512],
                            ident[w0:w0 + 32, w0 + so:w0 + so + B],
                            xp_sb[m][w0:w0 + 32, :, hc, :],
                            start=True, stop=False,
                            tile_position=(w0, 32 * hc))

                    # transpose h(t-1) -> stationary chunks for this step
                    if h_prev is not None:
                        T = tps.tile([128, 128], BF, tag="T", name="T")
                        nc.tensor.transpose(
                            T[:, 0:112], h_prev[0:112, :], ident[0:112, 0:112])
                        hTs = st.tile([128, 112], BF, tag="hTs", name="hTs")
                        nc.vector.tensor_copy(hTs[:, 0:112], T[:, 0:112])
                    else:
                        hTs = hTs_prev

                    # h-part matmuls, gate-block order g, i, f, o
                    for G in range(4):
                        for k in range(4):
                            for hc in range(4):
                                nc.tensor.matmul(
                                    P[32 * hc:32 * hc + B,
                                      128 * G:128 * (G + 1)],
                                    hTs[:, 32 * k:32 * k + B],
                                    Wh_t[k][:, (G * 4 + hc) * 128:
                                            (G * 4 + hc + 1) * 128],
                                    start=False, stop=(k == 3),
                                    tile_position=(0, 32 * hc))

                    # activations (blocks: 0=g tanh, 1=i, 2=f, 3=o sigmoid)
                    s_g = ch.tile([112, 128], FP, tag="sg", name="sg")
                    nc.scalar.activation(s_g[:, :], P[0:112, 0:128], AF.Tanh)
                    s_i = ch.tile([112, 128], FP, tag="si", name="si")
                    nc.scalar.activation(s_i[:, :], P[0:112, 128:256], AF.Sigmoid)
                    s_f = ch.tile([112, 128], FP, tag="sf", name="sf")
                    nc.scalar.activation(s_f[:, :], P[0:112, 256:384], AF.Sigmoid)
                    s_o = ch.tile([112, 128], FP, tag="so", name="so")
                    nc.scalar.activation(s_o[:, :], P[0:112, 384:512], AF.Sigmoid)

                    # cell/hidden update
                    t2 = ch.tile([112, 128], FP, tag="t2", name="t2")
                    nc.vector.tensor_mul(t2[:, :], s_i[:, :], s_g[:, :])
                    t1 = ch.tile([112, 128], FP, tag="t1", name="t1")
                    nc.vector.tensor_mul(t1[:, :], s_f[:, :], c_prev[:, :])
                    c_new = st.tile([112, 128], FP, tag="c", name="c")
                    nc.vector.tensor_add(c_new[:, :], t1[:, :], t2[:, :])
                    th = ch.tile([112, 128], FP, tag="th", name="th")
                    nc.scalar.activation(th[:, :], c_new[:, :], AF.Tanh)
                    h_new = st.tile([112, 128], BF, tag="h", name="h")
                    nc.vector.tensor_mul(h_new[:, :], s_o[:, :], th[:, :])

                    nc.sync.dma_start(out_h[t, :, :], h_new[:, :])

                    c_prev = c_new
                    h_prev = h_new
    nc.compile()
    return nc


def _host_prepare(x_full, weights, direction, bslice):
    xs = x_full[bslice]
    if direction == "bw":
        xs = xs[:, ::-1, :]
    xT = np.ascontiguousarray(xs.transpose(2, 1, 0).reshape(D, TOK))
    Wc = np.concatenate(
        [np.asarray(weights[f"W_{direction}_{n}"]).T for n in GATES], axis=1)
    bc = np.concatenate(
        [np.asarray(weights[f"b_{direction}_{n}"]) for n in GATES])[None, :]
    bf = ml_dtypes.bfloat16
    return {"xT": xT.astype(bf),
            "Wx": np.ascontiguousarray(Wc[:D]).astype(bf),
            "Wh": np.ascontiguousarray(Wc[D:]).astype(bf),
            "bias": np.ascontiguousarray(bc).astype(bf)}


def prepare(inputs):
    """Build (cached) the bass program and the 8 per-core input maps."""
    global _CACHED_NC
    inputs = {k: np.asarray(v) for k, v in inputs.items()}
    x = inputs["x"]
    Bx, Lx, _ = x.shape
    assert (Bx, Lx) == (64, L)

    if _CACHED_NC is None:
        _CACHED_NC = _build()
    nc = _CACHED_NC

    in_maps = []
    for ci in range(8):
        d = "fw" if ci < 4 else "bw"
        bs = (ci % 4) * B
        in_maps.append(_host_prepare(x, inputs, d, slice(bs, bs + B)))
    return nc, in_maps


def _unshard_core(oh):
    """out_h [L, 112, 128] bf16 -> [L, 16, 512] f32."""
    a = np.asarray(oh).astype(np.float32)
    parts = [a[:, 32 * hc:32 * hc + B, :] for hc in range(4)]
    return np.concatenate(parts, axis=2)  # [L, B, 512]


def kernel(**inputs):
    inputs = {k: np.asarray(v) for k, v in inputs.items()}
    x = inputs["x"]
    Bx = x.shape[0]
    nc, in_maps = prepare(inputs)
    meta = [("fw" if ci < 4 else "bw", (ci % 4) * B) for ci in range(8)]

    res = run_bass_kernel_spmd(nc, in_maps, core_ids=list(range(8)))

    hf = np.zeros((L, Bx, H), np.float32)
    hb = np.zeros((L, Bx, H), np.float32)
    for ci in range(8):
        d, bs = meta[ci]
        oh = _unshard_core(res.results[ci]["out_h"])  # (L, 16, H) time-major
        if d == "fw":
            hf[:, bs:bs + B, :] = oh
        else:
            hb[:, bs:bs + B, :] = oh[::-1]

    # faithful to the reference: stack time-major, flatten, hstack, reshape
    flat = np.concatenate([hf.reshape(-1, H), hb.reshape(-1, H)], axis=1)
    return flat.reshape(Bx, L, 2 * H).astype(np.float32)


# revision 11
# speedup vs baseline: 1.4497x; 1.4497x over previous
"""BiLSTM (B=64, L=256, D=512, H=512) on 8 Trainium2 NeuronCores.

Sharding: 8 cores = 2 directions x 4 batch-slices of 16 (weights replicated
per direction, sequential time loop local to each core).  Backward-direction
cores receive time-reversed x, so every core runs the identical SPMD program.

v3 design:
  - all matmuls bf16 (1 cycle/moving-row, single pass).
  - gate layout: TWO psum banks per step, Pfo=[f,o] and Pgi=[g,i], each
    [128, 256] with gate blocks of 128 cols; block = [4 h-chunks x
    32-partition groups, 128 free] (partition = 32*hc + b, free = h%128).
    Two banks -> the ACT tail for [f,o] starts while [g,i] matmuls run.
  - x-part GEMM is FOLDED into the recurrence loop (one (m,n) chunklet
    every 2 steps + 16-chunklet prologue): keeps the PE busy enough that
    the HAM clock gate stays at 2.4 GHz (v2 ran at 1.2 GHz 88% of the
    time), and hides the whole phase-1 cost in tail-wait PE idle slots.
  - transpose factoring: hT = T(sigma_o) * T(tanh c) elementwise; the two
    PE transposes run inside the tail and ONE DVE mul yields the bf16
    stationary for the next step directly.  out_h is written in the
    transposed layout [hl, 32k+b]; the host unscrambles.
  - per-step PE order: h-mm(fo) | h-mm(gi) | inject(t+1) | chunklet |
    T_o | T_th  -- inject/chunklet fill the ACT/DVE tail wait.
"""

import numpy as np
import ml_dtypes

from concourse import tile, mybir, bacc
from concourse.bass_utils import run_bass_kernel_spmd
from concourse.masks import make_identity

FP = mybir.dt.float32
BF = mybir.dt.bfloat16
AF = mybir.ActivationFunctionType

B = 16        # local batch per core
L = 256       # timesteps
D = 512       # input dim
H = 512       # hidden
NG = 4 * H    # gate width
TOK = L * B   # tokens per core
NM = TOK // 128  # 32 x-part token tiles

# gate order on device: blocks [g, i, f, o]; banks: gi = blocks 0:2,
# fo = blocks 2:4
GATES = "gifo"

N_PRO = 16    # prologue chunklets (4 m-tiles)
XPR_BUFS = 6  # xp ring depth in m-tiles

_CACHED_NC = None


def _build():
    nc = bacc.Bacc("TRN2", target_bir_lowering=False, debug=False)

    xT = nc.dram_tensor("xT", [D, TOK], BF, kind="ExternalInput").ap()
    Wx = nc.dram_tensor("Wx", [D, NG], BF, kind="ExternalInput").ap()
    Wh = nc.dram_tensor("Wh", [H, NG], BF, kind="ExternalInput").ap()
    bias = nc.dram_tensor("bias", [1, NG], BF, kind="ExternalInput").ap()
    out_h = nc.dram_tensor("out_h", [L, 128, 112], BF, kind="ExternalOutput").ap()

    with tile.TileContext(nc, trace_sim=False) as tc:
        with tc.tile_pool(name="wpool", bufs=1) as wpool, \
             tc.tile_pool(name="cpool", bufs=1) as cpool, \
             tc.tile_pool(name="xpr", bufs=XPR_BUFS) as xpr, \
             tc.tile_pool(name="p1x", bufs=3) as p1x, \
             tc.tile_pool(name="p1ps", bufs=2, space="PSUM") as p1ps, \
             tc.tile_pool(name="st", bufs=2) as st, \
             tc.tile_pool(name="ch", bufs=2) as ch, \
             tc.tile_pool(name="gps_fo", bufs=2, space="PSUM") as gps_fo, \
             tc.tile_pool(name="gps_gi", bufs=2, space="PSUM") as gps_gi, \
             tc.tile_pool(name="tps", bufs=1, space="PSUM") as tps:

            # ---- persistent weights / identity ----
            Wx_t, Wh_t = [], []
            for k in range(4):
                wt = wpool.tile([128, NG], BF, tag=f"wx{k}", name=f"wx{k}")
                nc.sync.dma_start(wt[:], Wx[128 * k:128 * (k + 1), :])
                Wx_t.append(wt)
            for k in range(4):
                wt = wpool.tile([128, NG], BF, tag=f"wh{k}", name=f"wh{k}")
                nc.sync.dma_start(wt[:], Wh[128 * k:128 * (k + 1), :])
                Wh_t.append(wt)
            bias_t = wpool.tile([1, NG], BF)
            nc.sync.dma_start(bias_t[:], bias[:, :])
            ones_t = cpool.tile([1, 128], BF)
            nc.vector.memset(ones_t[:, :], 1.0)
            ident = cpool.tile([128, 128], BF)
            make_identity(nc, ident[:, :])

            # ---- gate psum ring buffers, kept by step parity ----
            P_fo, P_gi = [None, None], [None, None]
            for z in range(2):
                P_fo[z] = gps_fo.tile([128, 256], FP, tag="Pfo", name=f"Pfo{z}")
                nc.vector.memset(P_fo[z][:, :], 0.0)
                P_gi[z] = gps_gi.tile([128, 256], FP, tag="Pgi", name=f"Pgi{z}")
                nc.vector.memset(P_gi[z][:, :], 0.0)

            # ---- x-part chunklet machinery (folded phase 1) ----
            xp_tiles = {}       # m -> 4D tile [128 tok, 4 G, 4 hc, 128]
            state = {"c": 0, "xm": None}

            def emit_chunklet():
                c = state["c"]
                if c >= 4 * NM:
                    return
                state["c"] += 1
                m, n = divmod(c, 4)
                if n == 0:
                    xm = p1x.tile([128, 4, 128], BF, tag="xm", name="xm")
                    for k in range(4):
                        nc.sync.dma_start(
                            xm[:, k, :],
                            xT[128 * k:128 * (k + 1), 128 * m:128 * (m + 1)])
                    xp_tiles[m] = xpr.tile(
                        [128, 4, 4, 128], BF, tag="xpr", name=f"xp{m}")
                    state["xm"] = xm
                xm = state["xm"]
                ps = p1ps.tile([128, 512], FP, tag="ps1", name="ps1")
                for k in range(4):
                    nc.tensor.matmul(
                        ps[:, :], xm[:, k, :],
                        Wx_t[k][:, 512 * n:512 * (n + 1)],
                        start=(k == 0), stop=False)
                nc.tensor.matmul(
                    ps[:, :], ones_t[:, :], bias_t[:, 512 * n:512 * (n + 1)],
                    start=False, stop=True)
                dst = xp_tiles[m][:, n, :, :]
                if c % 2 == 0:
                    nc.scalar.copy(dst, ps[:, :])
                else:
                    nc.vector.tensor_copy(dst, ps[:, :])

            def emit_inject(t):
                """x-part injection for step t.  start=True clears the whole
                32-partition column group's has_written bits, so exactly ONE
                start mm per (bank, column-group)."""
                m, r = divmod(t, 8)
                w0 = 32 * (r // 2)
                so = B * (r % 2)
                xp4 = xp_tiles[m]
                for gl, Pb in ((2, P_fo[t % 2]), (0, P_gi[t % 2])):
                    for hc in range(4):
                        nc.tensor.matmul(
                            Pb[32 * hc:32 * hc + B, 0:256],
                            ident[w0:w0 + 32, w0 + so:w0 + so + B],
                            xp4[w0:w0 + 32, gl:gl + 2, hc, :],
                            start=True, stop=False,
                            tile_position=(w0, 32 * hc))

            # ---- zero-init state ----
            c_prev = st.tile([112, 128], FP, tag="c", name="c0")
            nc.vector.memset(c_prev[:, :], 0.0)
            hTs_prev = st.tile([128, 112], BF, tag="hTs", name="hTs0")
            nc.vector.memset(hTs_prev[:, :], 0.0)

            # ---- prologue: first chunklets + inject(0) ----
            for _ in range(N_PRO):
                emit_chunklet()
            emit_inject(0)

            for t in range(L):
                Pfo = P_fo[t % 2]
                Pgi = P_gi[t % 2]

                # h-part matmuls: bank fo first (feeds the critical chain)
                for gl, Pb in ((2, Pfo), (0, Pgi)):
                    for k in range(4):
                        for Gb in range(2):
                            for hc in range(4):
                                nc.tensor.matmul(
                                    Pb[32 * hc:32 * hc + B,
                                       128 * Gb:128 * (Gb + 1)],
                                    hTs_prev[:, 32 * k:32 * k + B],
                                    Wh_t[k][:, ((gl + Gb) * 4 + hc) * 128:
                                            ((gl + Gb) * 4 + hc + 1) * 128],
                                    start=False, stop=(k == 3),
                                    tile_position=(0, 32 * hc))

                # PE filler work during this step's ACT/DVE tail
                if t + 1 < L:
                    emit_inject(t + 1)
                if t % 2 == 0:
                    emit_chunklet()

                # ---- tail ----
                # bank fo: one merged sigmoid over [f|o]
                s_fo = ch.tile([112, 256], BF, tag="sfo", name="sfo")
                nc.scalar.activation(s_fo[:, :], Pfo[0:112, :], AF.Sigmoid)
                t1 = ch.tile([112, 128], FP, tag="t1", name="t1")
                nc.vector.tensor_mul(t1[:, :], s_fo[:, 0:128], c_prev[:, :])
                # bank gi
                s_g = ch.tile([112, 128], BF, tag="sg", name="sg")
                nc.scalar.activation(s_g[:, :], Pgi[0:112, 0:128], AF.Tanh)
                s_i = ch.tile([112, 128], BF, tag="si", name="si")
                nc.scalar.activation(s_i[:, :], Pgi[0:112, 128:256], AF.Sigmoid)
                t2 = ch.tile([112, 128], FP, tag="t2", name="t2")
                nc.vector.tensor_mul(t2[:, :], s_i[:, :], s_g[:, :])
                c_new = st.tile([112, 128], FP, tag="c", name="c")
                nc.vector.tensor_add(c_new[:, :], t1[:, :], t2[:, :])
                th = ch.tile([112, 128], BF, tag="th", name="th")
                nc.scalar.activation(th[:, :], c_new[:, :], AF.Tanh)

                # transposes (PE) + one DVE mul -> next stationary + output.
                # DVE reads at most one PSUM operand, so T(sigma_o) is staged
                # to SBUF early (off the critical chain, during the gi ACTs).
                TT = tps.tile([128, 224], BF, tag="TT", name="TT")
                nc.tensor.transpose(
                    TT[:, 0:112], s_fo[0:112, 128:256], ident[0:112, 0:112])
                To_sb = ch.tile([128, 112], BF, tag="To", name="To")
                nc.vector.tensor_copy(To_sb[:, :], TT[:, 0:112])
                nc.tensor.transpose(
                    TT[:, 112:224], th[0:112, :], ident[0:112, 0:112])
                hTs_new = st.tile([128, 112], BF, tag="hTs", name="hTs")
                nc.vector.tensor_mul(
                    hTs_new[:, :], To_sb[:, :], TT[:, 112:224])

                nc.sync.dma_start(out_h[t, :, :], hTs_new[:, :])

                c_prev = c_new
                hTs_prev = hTs_new
    nc.compile()
    return nc


def _host_prepare(x_full, weights, direction, bslice):
    xs = x_full[bslice]
    if direction == "bw":
        xs = xs[:, ::-1, :]
    xT = np.ascontiguousarray(xs.transpose(2, 1, 0).reshape(D, TOK))
    Wc = np.concatenate(
        [np.asarray(weights[f"W_{direction}_{n}"]).T for n in GATES], axis=1)
    bc = np.concatenate(
        [np.asarray(weights[f"b_{direction}_{n}"]) for n in GATES])[None, :]
    bf = ml_dtypes.bfloat16
    return {"xT": xT.astype(bf),
            "Wx": np.ascontiguousarray(Wc[:D]).astype(bf),
            "Wh": np.ascontiguousarray(Wc[D:]).astype(bf),
            "bias": np.ascontiguousarray(bc).astype(bf)}


def prepare(inputs):
    """Build (cached) the bass program and the 8 per-core input maps."""
    global _CACHED_NC
    inputs = {k: np.asarray(v) for k, v in inputs.items()}
    x = inputs["x"]
    Bx, Lx, _ = x.shape
    assert (Bx, Lx) == (64, L)

    if _CACHED_NC is None:
        _CACHED_NC = _build()
    nc = _CACHED_NC

    in_maps = []
    for ci in range(8):
        d = "fw" if ci < 4 else "bw"
        bs = (ci % 4) * B
        in_maps.append(_host_prepare(x, inputs, d, slice(bs, bs + B)))
    return nc, in_maps


def _unshard_core(oh):
    """out_h [L, 128, 112] bf16 (transposed h) -> [L, 16, 512] f32.
    h[t, b, 128*k + hl] = oh[t, hl, 32*k + b]."""
    a = np.asarray(oh).astype(np.float32)
    parts = [a[:, :, 32 * k:32 * k + B].transpose(0, 2, 1) for k in range(4)]
    return np.concatenate(parts, axis=2)  # [L, B, 512]


def kernel(**inputs):
    inputs = {k: np.asarray(v) for k, v in inputs.items()}
    x = inputs["x"]
    Bx = x.shape[0]
    nc, in_maps = prepare(inputs)
    meta = [("fw" if ci < 4 else "bw", (ci % 4) * B) for ci in range(8)]

    res = run_bass_kernel_spmd(nc, in_maps, core_ids=list(range(8)))

    hf = np.zeros((L, Bx, H), np.float32)
    hb = np.zeros((L, Bx, H), np.float32)
    for ci in range(8):
        d, bs = meta[ci]
        oh = _unshard_core(res.results[ci]["out_h"])  # (L, 16, H) time-major
        if d == "fw":
            hf[:, bs:bs + B, :] = oh
        else:
            hb[:, bs:bs + B, :] = oh[::-1]

    # faithful to the reference: stack time-major, flatten, hstack, reshape
    flat = np.concatenate([hf.reshape(-1, H), hb.reshape(-1, H)], axis=1)
    return flat.reshape(Bx, L, 2 * H).astype(np.float32)
